# revision 25
# baseline (speedup 1.0000x reference)
"""Multi-head attention (AttnProcessor2_0) on 8 TRN2 NeuronCores.

Problem: B=2, S=4096, C=640, H=10, Dh=64.
  q/k/v = hs @ W{q,k,v}.T ; per-head scores = q k^T / 8 ; softmax ;
  out = probs v ; y = out @ Wo.T + b_out + hs

Sharding (no collectives): core c -> batch b=c//4, query block g=c%4
(1024 queries).  Each core recomputes full K/V for its batch (head-dim
on partitions), computes its own S/4 x S attention block, output
projection, bias+residual.  Host passes hidden states TRANSPOSED and
ROLLED by the query offset so the same SPMD program works on every
core (softmax+PV are permutation-invariant along the key axis).

fp8 pipeline (rel-err budget is huge: the residual passthrough
dominates ||y||, attention contributes ~0.5%):
  hsT, Wq*16, Wk*16, Wv*16 in fp8-e4m3 (weights scaled x16 into the
  e4m3 normal range; scores come out 256x, absorbed by the exp scale;
  v comes out 16x, absorbed by woT/16 host-side).
  Projections run fp8 DoubleRow (contraction chunk pairs (0,1),(2,3)
  at 2 elem/cycle + single chunk 4) -> ~1.6x faster than bf16.
  probs are stored fp8 (ACT exp writes e4m3 directly; the DVE
  Schraudolph offload writes e4m3 BITS via an int8 round) and PV runs
  DoubleRow over 256-key chunks with the v tile laid out
  [128, head, ktile, 65] -> 2x the old PV throughput.
QK stays bf16 (kT/qT hold 16k/16q), head-paired row-tiled matmuls.

Engine balance: ACT keeps ~60% of the 41.9M exps; the rest go to the
DVE as a one-pass Schraudolph bit-trick (uniform bias cancels in the
softmax ratio).  The softmax reciprocal moved to ACT as ln/exp read
straight from the PSUM denominator row (kills the 30us DVE reciprocal
and 20us of drain copies).
"""

import sys

if "/opt/trn_rl_repo" not in sys.path:
    sys.path.insert(0, "/opt/trn_rl_repo")

from collections import deque
from contextlib import ExitStack

import ml_dtypes
import numpy as np

import concourse.bass as bass
import concourse.tile as tile
from concourse import mybir
from concourse.bass import ts

BF16 = mybir.dt.bfloat16
F32 = mybir.dt.float32
F8 = mybir.dt.float8e4
I8 = mybir.dt.int8
DR = mybir.MatmulPerfMode.DoubleRow

B, S, C = 2, 4096, 640
H, DH = 10, 64
NCORES = 8
GROUP = 4  # cores per batch element
SQ = S // GROUP  # 1024 queries per core
SCALE = 0.125  # 1/sqrt(64)
WS = 16.0  # host-side weight scale for fp8 Wq/Wk/Wv
ESCALE = SCALE / (WS * WS)  # exp scale applied to the raw (256x) scores
CCH = C // 128  # 5 feature chunks (2 heads each)
NJT = S // 512  # 8 key tiles for K proj
NJC = S // 128  # 32 key chunks for attention
NJP = NJC // 2  # 16 double-key (256) chunks for DoubleRow PV
NIT = SQ // 512  # 2 query tiles
VST = DH + 1  # 65: used columns per (head, ktile) in v tiles (ones col appended)
VPAD = 80  # allocated stride: DoubleRow LDWEIGHTS needs the ktile step %16==0

# Schraudolph exp offload: selected score chunks compute exp on the DVE
# as an fp8 bit-trick (one tensor_scalar: bits = round(s*A + B) written
# as int8 and read back as e4m3 gives 2^(s*ESCALE*log2e) with a few %
# per-element jitter and ~zero mean; any constant bias cancels in the
# softmax ratio).  This moves work off the bottleneck ScalarE onto the
# DVE.  Set empty to disable.
LN2 = float(np.log(2.0))
SCHRAUD_A = ESCALE * 8.0 / LN2
SCHRAUD_B = 7.0 * 8.0 - 0.4656
# Block-end choreography: the next block's first QK reuses the sc PSUM
# buffer of jc30, and its second QK that of jc31.  jc30 goes to the DVE
# (short queue -> fast release) while 27-29,31 stay on ACT; the rawp
# drain then only queues behind TS(30) on the DVE and the LN only
# behind exp(31) on ACT, so the single pvb buffer also frees in time.
OFFLOAD_JC = frozenset({1, 3, 5, 7, 9, 11, 13, 15, 17, 19, 21, 24, 26, 30})


def build_nc() -> bass.Bass:
    nc = bass.Bass()
    hsT = nc.declare_dram_parameter("hsT", [C, S], F8, isOutput=False)
    res = nc.declare_dram_parameter("res", [C, SQ], F32, isOutput=False)
    wqT = nc.declare_dram_parameter("wqT", [C, C], F8, isOutput=False)
    wkT = nc.declare_dram_parameter("wkT", [C, C], F8, isOutput=False)
    wvT = nc.declare_dram_parameter("wvT", [C, C], F8, isOutput=False)
    woT = nc.declare_dram_parameter("woT", [C, C], BF16, isOutput=False)
    out = nc.declare_dram_parameter("out", [C, SQ], F32, isOutput=True)

    with ExitStack() as ctx:
        tc = ctx.enter_context(tile.TileContext(nc))
        # outer pool: tensors whose lifetime spans projections AND attention
        sb = ctx.enter_context(tc.tile_pool(name="sb", bufs=1))

        kT_sb = [sb.tile([128, S], BF16, tag=f"kT{i}", name=f"kT{i}") for i in range(CCH)]
        # head-pair q: rows 0:64 = even head, 64:128 = odd head.  The QK
        # matmuls are K=64 row-tiled (tile_position (0,0)/(64,0)) and run
        # concurrently in the PE array -- no zero padding needed.
        qT_sb = [sb.tile([128, SQ], BF16, tag=f"qT{i}", name=f"qT{i}") for i in range(CCH)]
        # v tiles: one per 256-key chunk, fp8, [128, head, ktile, 80(65 used)]
        # (ones col at x=64 turns the PV matmul into numerator+denominator)
        v_sb = [sb.tile([128, H * 2 * VPAD], F8, tag=f"v{j}", name=f"v{j}")
                for j in range(NJP)]
        ones_sb = sb.tile([1, 512], BF16, tag="ones", name="ones")
        nc.vector.memset(ones_sb[:], 1.0)

        # prefetch the exp/ln table set while DMAs stream (the pseudo
        # ACT_TABLE_LOAD walrus inserts before the first real exp would
        # otherwise land on the critical path, ~1.3us).  First DVE +
        # ACT instructions of the kernel so nothing delays the load.
        warm = sb.tile([1, 16], F32, tag="warm", name="warm")
        nc.vector.memset(warm[:], 0.0)
        nc.scalar.activation(warm[:], warm[:],
                             mybir.ActivationFunctionType.Exp,
                             bias=0.0, scale=0.0)

        # ---------------- load + first projections ----------------
        # Each input tensor is ONE wide SBUF tile filled by ONE DMA (the
        # Sync engine issues triggers at ~600ns each -- 20 small DMAs
        # serialized the old startup).  Chunk cc of a tensor lives at
        # free-offset cc*width; h3/wk3/... are [128, chunk, width] views.
        load = ctx.enter_context(tc.tile_pool(name="load", bufs=1))
        hsT_big = load.tile([128, CCH * S], F8, tag="hsT", name="hsT")
        h3 = hsT_big[:].rearrange("p (f s) -> p f s", s=S)
        wk3 = load.tile([128, CCH * C], F8, tag="wk", name="wk")[:] \
            .rearrange("p (f c) -> p f c", c=C)
        wq3 = load.tile([128, CCH * C], F8, tag="wq", name="wq")[:] \
            .rearrange("p (f c) -> p f c", c=C)
        wv3 = load.tile([128, CCH * C], F8, tag="wv", name="wv")[:] \
            .rearrange("p (f c) -> p f c", c=C)
        # full Wo resident: kills the per-oproj weight DMAs; with
        # head-paired attn the contraction is all-real
        wo3 = load.tile([128, CCH * C], BF16, tag="wo", name="wo")[:] \
            .rearrange("p (f c) -> p f c", c=C)
        # staged startup: the first DoubleRow matmul of the first K-proj
        # stripe needs only wk chunks 0-1 and h3[:, 0:2, 0:512] -- land
        # those first so the PE starts as early as possible
        nc.sync.dma_start(
            wk3[:, 0:2, :], wkT[0:256, :].rearrange("(f p) c -> p f c", p=128)
        )
        nc.sync.dma_start(
            h3[:, 0:2, 0:512],
            hsT[0:256, 0:512].rearrange("(f p) s -> p f s", p=128),
        )
        nc.sync.dma_start(
            wk3[:, 2:5, :], wkT[256:640, :].rearrange("(f p) c -> p f c", p=128)
        )
        nc.sync.dma_start(
            h3[:, 2:5, 0:512],
            hsT[256:640, 0:512].rearrange("(f p) s -> p f s", p=128),
        )
        nc.sync.dma_start(
            h3[:, :, 512:SQ],
            hsT[:, 512:SQ].rearrange("(f p) s -> p f s", p=128),
        )
        nc.sync.dma_start(wq3, wqT[:, :].rearrange("(f p) c -> p f c", p=128))
        nc.sync.dma_start(wv3, wvT[:, :].rearrange("(f p) c -> p f c", p=128))
        nc.sync.dma_start(wo3, woT[:, :].rearrange("(f p) c -> p f c", p=128))

        def emit_hsT_tail():
            # deferred until after the first exp so ScalarE's conservative
            # vector-clock waits don't cover this DMA
            for blk in range(SQ, S, SQ):
                nc.sync.dma_start(
                    h3[:, :, blk : blk + SQ],
                    hsT[:, blk : blk + SQ].rearrange("(f p) s -> p f s", p=128),
                )

        def proj_ops(w3, dst, dc, jt, pool):
            # one K/Q projection stripe as 4 micro-ops (2 DoubleRow MMs +
            # 1 single + cast) so the background drain never inserts more
            # than ~2 matmuls between attention-stream matmuls
            st = {}

            def mm(cs, width, start, stop):
                def f():
                    if "ps" not in st:
                        st["ps"] = pool.tile([128, 512], F32, tag="pp",
                                             name="pp", bufs=2)
                    if width == 2:
                        nc.tensor.matmul(
                            st["ps"][:],
                            w3[:, cs : cs + 2, ts(dc, 128)],
                            h3[:, cs : cs + 2, ts(jt, 512)],
                            start=start,
                            stop=stop,
                            perf_mode=DR,
                        )
                    else:
                        nc.tensor.matmul(
                            st["ps"][:],
                            w3[:, cs, ts(dc, 128)],
                            h3[:, cs, ts(jt, 512)],
                            start=start,
                            stop=stop,
                        )
                return f

            def cast():
                nc.vector.tensor_copy(dst[:, ts(jt, 512)], st["ps"][:])

            return [mm(0, 2, True, False), mm(2, 2, False, False),
                    mm(4, 1, False, True), cast]

        def emit_kproj(dc, jt, pool):
            for f in proj_ops(wk3, kT_sb[dc], dc, jt, pool):
                f()

        def emit_qproj(dc, it, pool):
            for f in proj_ops(wq3, qT_sb[dc], dc, it, pool):
                f()

        def emit_vproj(jc, pool):
            jp, t = divmod(jc, 2)
            vt = v_sb[jp]
            v4 = vt[:].rearrange("p (h t x) -> p h t x", t=2, x=VPAD)
            for d0, dn in ((0, 512), (512, 128)):
                ps = pool.tile([128, 512], F32, tag="pp", name="pp", bufs=2)
                nc.tensor.matmul(
                    ps[:, 0:dn],
                    h3[:, 0:2, ts(jc, 128)],
                    wv3[:, 0:2, d0 : d0 + dn],
                    start=True, stop=False, perf_mode=DR,
                )
                nc.tensor.matmul(
                    ps[:, 0:dn],
                    h3[:, 2:4, ts(jc, 128)],
                    wv3[:, 2:4, d0 : d0 + dn],
                    start=False, stop=False, perf_mode=DR,
                )
                nc.tensor.matmul(
                    ps[:, 0:dn],
                    h3[:, 4, ts(jc, 128)],
                    wv3[:, 4, d0 : d0 + dn],
                    start=False, stop=True,
                )
                nc.vector.tensor_copy(
                    v4[:, d0 // DH : (d0 + dn) // DH, t, 0:DH],
                    ps[:, 0:dn].rearrange("p (h x) -> p h x", x=DH),
                )

        # ones columns of all v tiles set once up front (DVE is idle
        # during the DMA-bound startup)
        for jp in range(NJP):
            v4c = v_sb[jp][:].rearrange("p (h t x) -> p h t x", t=2, x=VPAD)
            nc.vector.memset(v4c[:, :, :, DH : DH + 1], 1.0)

        with tc.tile_pool(name="pp0", bufs=2, space="PSUM") as pp0:
            # spin the PE while the startup DMAs land: the HAM clock gate
            # releases the full 2.4GHz only after ~3.4us of sustained
            # activity, so burn the DMA wait on dummy rank-1 matmuls
            wps = pp0.tile([DH, 512], F32, tag="wmm", name="wmm", bufs=1)
            for _ in range(8):
                nc.tensor.matmul(wps[:], ones_sb[0:1, 0:DH], ones_sb[0:1, :],
                                 start=True, stop=True)
            for jt in range(2):
                emit_kproj(0, jt, pp0)
            emit_qproj(0, 0, pp0)

        # ---------------- attention phase ----------------
        # attn2[hp]: head pair packed (rows 0:64 even head, 64:128 odd) --
        # the output projection contracts all 128 rows with no padding
        attn2_sb = [sb.tile([128, SQ], BF16, tag=f"attn{p}", name=f"attn{p}")
                    for p in range(CCH)]
        with tc.tile_pool(name="ap", bufs=1, space="PSUM") as ap, \
             tc.tile_pool(name="pt", bufs=6) as pt_pool, \
             tc.tile_pool(name="ob", bufs=3) as ob, \
             tc.tile_pool(name="scratch", bufs=3) as scratch:
            def norm_dve(hp, pvb, p_isl):
                # drain both heads' pv numerators into one packed bf16
                # tile (frees the pv PSUM banks for the next pair);
                # reciprocal of the denominator row via ACT ln + exp(-x)
                # read straight from PSUM (the DVE iterative reciprocal
                # is 8 cyc/elem and was eating 30us+20us of copies)
                rawp = scratch.tile([128, 512], BF16, tag="raw", name="raw",
                                    bufs=2)
                nc.vector.tensor_copy(rawp[0:DH, :], pvb[0:DH, 0:512])
                nc.vector.tensor_copy(rawp[DH:128, :], pvb[0:DH, 512:1024])
                lg = scratch.tile([1, 1024], F32, tag="lg", name="lg", bufs=2)
                nc.scalar.activation(
                    lg[:], pvb[DH : DH + 1, :],
                    mybir.ActivationFunctionType.Ln, bias=0.0, scale=1.0,
                )
                rc = scratch.tile([1, 1024], BF16, tag="rc", name="rc",
                                  bufs=2)
                with nc.allow_low_precision(reason="softmax recip bf16"):
                    nc.scalar.activation(
                        rc[:], lg[:],
                        mybir.ActivationFunctionType.Exp,
                        bias=0.0, scale=-1.0,
                    )
                return (hp, p_isl, rc, rawp)

            def norm_pe(state, anchor, pool):
                # two rank-1 PE broadcasts of the reciprocals (column
                # tiles (0,0) and (0,64)), pinned behind the anchor QK so
                # the recip latency is hidden
                hp, p_isl, rc, rawp = state
                ps = pool.tile([128, 512], F32, tag="pp", name="pp", bufs=2)
                r_mm = nc.tensor.matmul(
                    ps[0:DH, :],
                    ones_sb[0:1, 0:DH],
                    rc[0:1, 0:512],
                    start=True,
                    stop=True,
                )
                nc.tensor.matmul(
                    ps[DH:128, :],
                    ones_sb[0:1, 0:DH],
                    rc[0:1, 512:1024],
                    start=True,
                    stop=True,
                )
                if anchor is not None:
                    tile.add_dep_helper(
                        r_mm.ins, anchor.ins, sync=False,
                        reason="norm R after anchor QK (hide recip latency)",
                    )
                nc.vector.tensor_mul(
                    attn2_sb[hp][:, p_isl], rawp[:], ps[:]
                )

            def oproj_ops(ec, it, ps_tile=None):
                st = {"ps": ps_tile} if ps_tile is not None else {}

                def mm(r):
                    def f():
                        if "ps" not in st:
                            st["ps"] = ap.tile([128, 512], F32, tag="pp",
                                               name="pp", bufs=2)[:]
                        if "rt" not in st:
                            rt = ob.tile([128, 512], F32, tag="rt",
                                         name="rt", bufs=2)
                            nc.sync.dma_start(
                                rt[:], res[ts(ec, 128), ts(it, 512)]
                            )
                            st["rt"] = rt
                        nc.tensor.matmul(
                            st["ps"],
                            wo3[:, r, ts(ec, 128)],
                            attn2_sb[r][:, ts(it, 512)],
                            start=(r == 0),
                            stop=(r == CCH - 1),
                        )
                    return f

                def fin():
                    ot = ob.tile([128, 512], F32, tag="ot", name="ot", bufs=2)
                    nc.vector.tensor_add(ot[:], st["ps"], st["rt"][:])
                    nc.sync.dma_start(out[ts(ec, 128), ts(it, 512)], ot[:])

                return [mm(r) for r in range(CCH)] + [fin]

            def emit_oproj(ec, it):
                for f in oproj_ops(ec, it):
                    f()

            # Background work flows through a micro-op queue drained at
            # most 2 ops per jc step, so no more than ~2 weight matmuls
            # ever sit between attention-stream matmuls on the PE (a
            # whole projection burst stalls the exp pipeline).
            bgq = deque()

            pending = []
            for it in range(NIT):
                isl = ts(it, 512)
                for hp in range(CCH):
                    if it == 0:
                        for jt in range(2, NJT):
                            bgq.extend(proj_ops(wk3, kT_sb[hp], hp, jt, ap))
                        if hp < CCH - 1:
                            for jt in range(2):
                                bgq.extend(
                                    proj_ops(wk3, kT_sb[hp + 1], hp + 1, jt, ap)
                                )
                            bgq.extend(proj_ops(wq3, qT_sb[hp + 1], hp + 1, 0, ap))
                        else:
                            bgq.extend(proj_ops(wq3, qT_sb[0], 0, 1, ap))
                    else:
                        if hp == 0:
                            bgq.extend(proj_ops(wq3, qT_sb[1], 1, 1, ap))
                        elif hp == 1:
                            bgq.extend(oproj_ops(0, 0))
                            bgq.extend(oproj_ops(1, 0))
                            bgq.extend(proj_ops(wq3, qT_sb[2], 2, 1, ap))
                        elif hp == 2:
                            bgq.extend(oproj_ops(2, 0))
                            bgq.extend(proj_ops(wq3, qT_sb[3], 3, 1, ap))
                        elif hp == 3:
                            bgq.extend(oproj_ops(3, 0))
                            bgq.extend(proj_ops(wq3, qT_sb[4], 4, 1, ap))
                        else:
                            bgq.extend(oproj_ops(4, 0))
                    vtodo = {}
                    if it == 0 and hp == 0:
                        # V chunks 0..7 front-loaded (keys 0:1024 resident
                        # before the hsT tail lands), then one chunk per
                        # step four steps ahead of its PV use
                        for jc in range(4):
                            vtodo[jc] = [2 * jc, 2 * jc + 1]
                        for jc in range(4, 28):
                            vtodo[jc] = [jc + 4]
                    # both heads' PV accumulators in ONE 2-bank tile so
                    # the denominator row is a single [1,1024] ACT read
                    pvb = ap.tile([DH + 1, 1024], F32, tag="pv", bufs=1,
                                  name="pv")
                    pt2 = None
                    # PV matmuls run 2 chunks behind their probs: the PE
                    # queue is in-order, so a PV issued right after its
                    # pair's exp stalls the whole engine for ~1us per
                    # pair waiting on ACT/DVE.  Delaying the emission
                    # gives the exp pipeline time to land first.
                    pv_q = deque()
                    for jc in range(NJC):
                        jp, half = divmod(jc, 2)
                        sc = ap.tile([128, 1024], F32, tag="sc", bufs=2,
                                     name="sc")
                        qk0 = nc.tensor.matmul(
                            sc[:, 0:512],
                            kT_sb[hp][0:DH, ts(jc, 128)],
                            qT_sb[hp][0:DH, isl],
                            start=True,
                            stop=True,
                        )
                        nc.tensor.matmul(
                            sc[:, 512:1024],
                            kT_sb[hp][DH:128, ts(jc, 128)],
                            qT_sb[hp][DH:128, isl],
                            start=True,
                            stop=True,
                        )
                        if half == 0:
                            pt2 = pt_pool.tile([128, 2048], F8,
                                               tag="pt", name="pt")
                        dst = pt2[:, half * 1024 : (half + 1) * 1024]
                        # (0,0): DVE is busy with the front-loaded V casts
                        # until ~jc8.  Last block: push the final exps to
                        # the DVE so the tail norm chain isn't queued
                        # behind them on ACT.
                        offl = jc in OFFLOAD_JC
                        if it == 0 and hp == 0:
                            offl = offl and jc >= 9
                        elif it == NIT - 1 and hp == CCH - 1:
                            offl = offl or jc in (29, 31)
                        if offl:
                            nc.vector.tensor_scalar(
                                out=dst.bitcast(I8),
                                in0=sc[:],
                                scalar1=SCHRAUD_A,
                                scalar2=SCHRAUD_B,
                                op0=mybir.AluOpType.mult,
                                op1=mybir.AluOpType.add,
                            )
                        else:
                            nc.scalar.activation(
                                dst, sc[:],
                                mybir.ActivationFunctionType.Exp,
                                bias=0.0, scale=ESCALE,
                            )
                        if it == 0 and hp == 0 and jc == 0:
                            emit_hsT_tail()
                        if pending and jc == 5:
                            norm_pe(pending.pop(0), qk0, ap)
                        for j in vtodo.get(jc, ()):
                            emit_vproj(j, ap)
                        for _ in range(2):
                            if bgq:
                                bgq.popleft()()
                        if half == 1:
                            p3 = pt2[:].rearrange("p (t q) -> p t q", t=2)
                            v4 = v_sb[jp][:].rearrange(
                                "p (h t x) -> p h t x", t=2, x=VPAD
                            )

                            def mk_pv(col0, head, p3=p3, v4=v4, jp=jp):
                                def f():
                                    nc.tensor.matmul(
                                        pvb[:, col0 : col0 + 512],
                                        v4[:, head, :, 0:VST],
                                        p3[:, :, col0 : col0 + 512],
                                        start=(jp == 0),
                                        stop=(jp == NJP - 1),
                                        perf_mode=DR,
                                    )
                                return f

                            # jp0 (start=True) is gated on the previous
                            # block's pvb drain (rawp+ln); give that
                            # chain extra slack and let the queue catch
                            # up at 2 drains/step
                            rdy = jc + 2 if jp > 0 else jc + 5
                            pv_q.append((rdy, mk_pv(0, 2 * hp)))
                            pv_q.append((rdy, mk_pv(512, 2 * hp + 1)))
                        drained = 0
                        while pv_q and pv_q[0][0] <= jc and drained < 2:
                            pv_q.popleft()[1]()
                            drained += 1
                    while pv_q:
                        pv_q.popleft()[1]()
                    pending.append(norm_dve(hp, pvb, isl))
            while bgq:
                bgq.popleft()()
            # tail: the final pair's attn2[4] gates only the r=4 matmul of
            # each output projection.  The sc ring is dead now, so its 4
            # PSUM banks hold the r=0..3 partial sums of four output
            # projections (plus one in a pp slot; the other pp slot stays
            # free for the norm's R) -- all five accumulate underneath
            # the final reciprocal chain, then only r=4 + the residual
            # add remain.
            sc_a = ap.tile([128, 1024], F32, tag="sc", name="sc", bufs=2)
            sc_b = ap.tile([128, 1024], F32, tag="sc", name="sc", bufs=2)
            tail_ps = [None, sc_a[:, 0:512], sc_a[:, 512:1024],
                       sc_b[:, 0:512], sc_b[:, 512:1024]]
            tail_ops = [oproj_ops(ec, 1, ps_tile=tail_ps[ec])
                        for ec in range(CCH)]
            for ec in range(CCH):
                for f in tail_ops[ec][0:4]:
                    f()
            for st in pending:
                norm_pe(st, None, ap)
            for ec in range(CCH):
                for f in tail_ops[ec][4:]:
                    f()

    _spill_matmul_waits(nc)
    return nc


# walrus embedded-sync-wait capacity per BIR opcode.  Matmult holds a
# single wait; excess waits hoist onto the paired Ldweights (in-order
# issue on PE makes that equivalent).  Other compute ops spill onto
# EventSemaphore carrier instructions inserted just before them on the
# same engine.  DMACopy / Drain / EventSemaphore handle many waits
# natively (bacc emits such itself) and are left alone.
_WAIT_CAPS = {
    "InstMatmult": 1,
    "InstLdweights": 1,
    "InstActivation": 1,
    "InstReciprocal": 1,
    "InstTensorTensor": 1,
    "InstTensorCopy": 1,
    "InstTensorScalarPtr": 1,
    "InstTensorReduce": 1,
    "InstMemset": 1,
    "InstDMACopy": 1,
    "InstDrain": 1,
    "InstCustomDveAnt": 1,
}
_ES_CAP = 2  # waits per EventSemaphore carrier (walrus: <=2 waits, <=1 update)


def _spill_matmul_waits(nc: bass.Bass) -> None:
    spill_id = [0]

    def carriers(excess, engine):
        out = []
        for i in range(0, len(excess), _ES_CAP):
            es = mybir.InstEventSemaphore(
                name=f"wait-spill-{spill_id[0]}", ins=[], outs=[]
            )
            spill_id[0] += 1
            es.engine = engine
            es.sync_info = mybir.SyncInfo(
                on_wait=excess[i : i + _ES_CAP], on_update=[]
            )
            out.append(es)
        return out

    for f in nc.m.functions:
        for blk in f.blocks:
            insts = blk.instructions
            i = 0
            while i < len(insts):
                inst = insts[i]
                tn = type(inst).__name__
                cap = _WAIT_CAPS.get(tn)
                si = inst.sync_info
                if cap is None or si is None or len(si.on_wait) <= cap:
                    i += 1
                    continue
                w = list(si.on_wait)
                if tn == "InstMatmult" and cap == 1:
                    # Keep the latest-satisfied dependency (an ACT- or
                    # DVE-produced operand, e.g. probs from exp or the
                    # Schraudolph tensor_scalar) embedded on the matmul;
                    # other LATE elementwise deps go on an EventSemaphore
                    # between the Ldweights and the matmul; only
                    # early-satisfied deps (casts, DMA, WAR) hoist onto
                    # the Ldweights itself -- a live wait on the LDW
                    # blocks its background weight prefetch and
                    # serializes ~100ns of DoubleRow weight-load into
                    # the matmul.
                    late = [x for x in w
                            if "Activation" in (x.ant_name or "")
                            or "TensorScalar" in (x.ant_name or "")]
                    if late:
                        keep = [late[-1]]
                        mid = [x for x in late if x is not late[-1]]
                        excess = [x for x in w if x not in late]
                    else:
                        keep, excess = w[-cap:], w[:-cap]
                        mid = []
                else:
                    keep, excess = w[-cap:], w[:-cap]
                    mid = []
                prev = insts[i - 1] if i > 0 else None
                if (
                    tn == "InstMatmult"
                    and prev is not None
                    and type(prev).__name__ == "InstLdweights"
                    and len(((prev.sync_info and prev.sync_info.on_wait) or []))
                    + len(excess) <= 1
                ):
                    psi = prev.sync_info
                    pw = list(psi.on_wait) if psi is not None else []
                    pu = list(psi.on_update) if psi is not None else []
                    prev.sync_info = mybir.SyncInfo(on_wait=pw + excess, on_update=pu)
                else:
                    mid = excess + mid
                if mid:
                    # carriers sit between the LDW and the MM: the LDW
                    # stays free to prefetch under the previous matmul
                    new = carriers(mid, inst.engine)
                    insts[i:i] = new
                    i += len(new)
                inst.sync_info = mybir.SyncInfo(
                    on_wait=keep, on_update=list(si.on_update)
                )
                i += 1


_CACHED_NC = None


def get_nc() -> bass.Bass:
    global _CACHED_NC
    if _CACHED_NC is None:
        _CACHED_NC = build_nc()
    return _CACHED_NC


def make_in_maps(hidden_states, Wq, Wk, Wv, Wo, b_out):
    hs = np.asarray(hidden_states, dtype=np.float32)
    bf = ml_dtypes.bfloat16
    f8 = ml_dtypes.float8_e4m3
    wqT = (np.ascontiguousarray(np.asarray(Wq, np.float32).T) * WS).astype(f8)
    wkT = (np.ascontiguousarray(np.asarray(Wk, np.float32).T) * WS).astype(f8)
    wvT = (np.ascontiguousarray(np.asarray(Wv, np.float32).T) * WS).astype(f8)
    woT = (np.ascontiguousarray(np.asarray(Wo, np.float32).T) / WS).astype(bf)
    bias = np.asarray(b_out, np.float32).reshape(C, 1)
    in_maps = []
    for c in range(NCORES):
        b, g = divmod(c, GROUP)
        i0 = g * SQ
        hsTb = hs[b].T  # [C, S]
        in_maps.append(
            {
                "hsT": np.ascontiguousarray(np.roll(hsTb, -i0, axis=1)).astype(f8),
                "res": np.ascontiguousarray(hsTb[:, i0 : i0 + SQ]) + bias,
                "wqT": wqT,
                "wkT": wkT,
                "wvT": wvT,
                "woT": woT,
            }
        )
    return in_maps


def assemble(results) -> np.ndarray:
    y = np.empty((B, S, C), np.float32)
    for c in range(NCORES):
        b, g = divmod(c, GROUP)
        i0 = g * SQ
        y[b, i0 : i0 + SQ, :] = np.asarray(results[c]["out"], np.float32).T
    return y


def kernel(**inputs) -> np.ndarray:
    from concourse.bass_utils import run_bass_kernel_spmd

    nc = get_nc()
    in_maps = make_in_maps(**inputs)
    res = run_bass_kernel_spmd(nc, in_maps, list(range(NCORES)))
    return assemble(res.results)


if __name__ == "__main__":
    import reference

    inputs = {k: np.asarray(v) for k, v in reference.setup_inputs().items()}
    got = kernel(**inputs)
    want = np.asarray(reference.reference(**inputs))
    err = np.linalg.norm(got - want) / np.linalg.norm(want)
    print("Relative error:", err)
